# revision 15
# baseline (speedup 1.0000x reference)
"""Trainium2 Bass kernel for a top-2 gated MoE layer (8 experts, H=1024, F=4096).

Strategy (expert parallelism across the 8 NeuronCores):
  - Host computes the top-2 routing (argsort of the fp32 gate logits) and
    gathers each expert's tokens into a padded, transposed activation block
    xgT [H, C] (C = padded per-expert capacity).  All heavy math runs on
    device; the host only shards/gathers.
  - Each core runs one expert: gate logits + top-2 softmax weights are
    recomputed on device from its gathered tokens, LayerNorm + fc1 + gelu +
    fc2 + bias + gate scaling all happen on device (matmuls in bf16 with
    fp32 PSUM accumulation, LN statistics/scalars in fp32).
  - Host scatter-adds the per-expert outputs back into the full [B,S,H]
    tensor.

Self-contained: shapes are hardcoded from the problem spec.
"""

import numpy as np
import ml_dtypes
from contextlib import ExitStack

TOP_K = 2
LN_EPS = 1e-5
B, S, H, E, F = 2, 2048, 1024, 8, 4096
T = B * S
P = 128
KH = H // P          # 8 H-tiles
FB = 1024            # F block size
NFB = F // FB        # 4 blocks
MF = FB // P         # 8 F-tiles per block

_BUILD_CACHE = {}


def _chunks(C):
    # Small first chunk so the LN -> fc1 pipeline fills quickly.
    out = []
    off = 0
    if C >= 768:
        out.append((0, 256))
        off = 256
    while C - off > 512:
        out.append((off, 512))
        off += 512
    if C - off:
        out.append((off, C - off))
    return out


def _build(C):
    """Build + compile the single-core Bass program (SPMD across 8 cores)."""
    if C in _BUILD_CACHE:
        return _BUILD_CACHE[C]

    import concourse.bass as bass  # noqa: F401
    import concourse.tile as tile
    import concourse.mybir as mybir
    from concourse import bacc, bass_isa

    bf = mybir.dt.bfloat16
    f32 = mybir.dt.float32
    AF = mybir.ActivationFunctionType
    OP = mybir.AluOpType

    nc = bacc.Bacc("TRN2", target_bir_lowering=False, debug=False, num_devices=8)

    d_xgT = nc.dram_tensor("xgT", [H, C], bf, kind="ExternalInput")
    d_w1 = nc.dram_tensor("w1", [H, F], bf, kind="ExternalInput")
    d_w2 = nc.dram_tensor("w2", [F, H], bf, kind="ExternalInput")
    d_wg = nc.dram_tensor("wg", [H, E], bf, kind="ExternalInput")
    d_b1 = nc.dram_tensor("b1", [F, 1], f32, kind="ExternalInput")
    d_b2 = nc.dram_tensor("b2", [H, 1], f32, kind="ExternalInput")
    d_lnw = nc.dram_tensor("lnw", [H, 1], f32, kind="ExternalInput")
    d_lnb = nc.dram_tensor("lnb", [H, 1], f32, kind="ExternalInput")
    d_he = nc.dram_tensor("he", [E, 1], f32, kind="ExternalInput")
    d_al = nc.dram_tensor("alpha8", [E, 1], f32, kind="ExternalInput")
    d_y = nc.dram_tensor("ytT", [H, C], f32, kind="ExternalOutput")

    chunks = _chunks(C)

    with tile.TileContext(nc) as tc, ExitStack() as ctx:
        const = ctx.enter_context(tc.tile_pool(name="const", bufs=1))
        gpool = ctx.enter_context(tc.tile_pool(name="gate", bufs=1))
        bpool = ctx.enter_context(tc.tile_pool(name="bcast", bufs=1))
        xpool = ctx.enter_context(tc.tile_pool(name="x", bufs=KH))
        sqpool = ctx.enter_context(tc.tile_pool(name="sq", bufs=3))
        tpool = ctx.enter_context(tc.tile_pool(name="t1", bufs=2))
        hpool = ctx.enter_context(tc.tile_pool(name="hdn", bufs=KH))
        w1pool = ctx.enter_context(tc.tile_pool(name="w1", bufs=9))
        w2pool = ctx.enter_context(tc.tile_pool(name="w2", bufs=9))
        apool = ctx.enter_context(tc.tile_pool(name="acts", bufs=8))
        ypool = ctx.enter_context(tc.tile_pool(name="yacc", bufs=KH))
        ps_small = ctx.enter_context(
            tc.tile_pool(name="ps_small", bufs=2, space="PSUM"))
        ps1 = ctx.enter_context(tc.tile_pool(name="ps1", bufs=3, space="PSUM"))
        ps2 = ctx.enter_context(tc.tile_pool(name="ps2", bufs=3, space="PSUM"))

        # ---- constants / small params ----
        ones_k = const.tile([P, 1], bf)
        nc.vector.memset(ones_k, 1.0)
        # PE warm-up: ~5us of junk matmuls trains the HAM clock gate to
        # 2.4 GHz while the first x DMAs are still in flight.
        warm_rhs = const.tile([P, 512], bf)
        nc.vector.memset(warm_rhs, 0.0)
        ps_w = ps_small.tile([1, 512], f32, tag="pss", name="warm")
        for i in range(16):
            nc.tensor.matmul(ps_w[:], ones_k[:], warm_rhs[:],
                             start=True, stop=True)
        wg_sb = const.tile([P, KH, E], bf)
        for k in range(KH):
            nc.sync.dma_start(wg_sb[:, k, :], d_wg.ap()[k * P:(k + 1) * P, :])
        lnw_sb = const.tile([P, KH], f32)
        lnb_sb = const.tile([P, KH], f32)
        b2_sb = const.tile([P, KH], f32)
        for (t_sb, dram) in ((lnw_sb, d_lnw), (lnb_sb, d_lnb), (b2_sb, d_b2)):
            nc.sync.dma_start(
                t_sb[:], dram.ap().rearrange("(t p) o -> p (t o)", p=P))
        b1_sb = const.tile([P, F // P], f32)
        nc.sync.dma_start(
            b1_sb[:], d_b1.ap().rearrange("(t p) o -> p (t o)", p=P))
        he_sb = const.tile([E, 1], f32)
        nc.sync.dma_start(he_sb[:], d_he.ap())
        al_sb = const.tile([E, 1], f32)
        nc.sync.dma_start(al_sb[:], d_al.ap())

        # ---- Phases A-C, pipelined along C-chunks so the PE can start the
        # fc1 matmuls of chunk 0 while later chunks are still in LN/gate ----
        eps_t = gpool.tile([1, 1], f32)
        nc.vector.memset(eps_t, float(LN_EPS))
        xk = [xpool.tile([P, C], bf, tag="xk", name=f"xk{k}")
              for k in range(KH)]
        hdn = [hpool.tile([P, C], bf, tag="hdn", name=f"hdn{k}")
               for k in range(KH)]
        sums = gpool.tile([1, C], f32)
        sumsq = gpool.tile([1, C], f32)
        varb = gpool.tile([1, C], f32)
        l_sb = gpool.tile([E, C], f32)
        m1b = gpool.tile([E, C], f32)
        eqm = gpool.tile([E, C], f32)
        m2t = gpool.tile([E, C], f32)
        comb_row = gpool.tile([1, C], f32)
        sums_b = bpool.tile([P, C], f32)
        inv_b = bpool.tile([P, C], f32)
        comb_b = bpool.tile([P, C], f32)

        # All x slices first (small), then F-block 0 weights: its fc1/fc2 are
        # emitted inside the chunk loop so the (in-order) PE never waits on
        # later chunks' LN pipeline.
        for ci, (off, w) in enumerate(chunks):
            for k in range(KH):
                nc.sync.dma_start(xk[k][:, off:off + w],
                                  d_xgT.ap()[k * P:(k + 1) * P, off:off + w])
        y_acc = [ypool.tile([P, C], f32, tag="yacc", name=f"yacc{h}")
                 for h in range(KH)]
        w1t0, w2t0 = [], []
        for k in range(KH):
            w1k = w1pool.tile([P, FB], bf, tag="w1", name=f"w1_0_{k}")
            nc.sync.dma_start(w1k[:], d_w1.ap()[k * P:(k + 1) * P, 0:FB])
            w1t0.append(w1k)
        for k in range(MF):
            w2k = w2pool.tile([P, H], bf, tag="w2", name=f"w2_0_{k}")
            nc.sync.dma_start(w2k[:], d_w2.ap()[k * P:(k + 1) * P, :])
            w2t0.append(w2k)
        at0 = [apool.tile([P, C], bf, tag="acts", name=f"a_0_{m}")
               for m in range(MF)]

        for ci, (off, w) in enumerate(chunks):
            sl = slice(off, off + w)
            # column sums / sums of squares / gate logits via PE reductions
            # (squares on GpSimd to keep the DVE free for the LN chain)
            ps_a = ps_small.tile([1, w], f32, tag="pss", name=f"ps_sum{off}")
            for k in range(KH):
                nc.tensor.matmul(ps_a[:], ones_k[:], xk[k][:, sl],
                                 start=(k == 0), stop=(k == KH - 1))
            nc.vector.tensor_copy(sums[:, sl], ps_a[:])
            ps_b = ps_small.tile([1, w], f32, tag="pss", name=f"ps_sq{off}")
            for k in range(KH):
                sq_c = sqpool.tile([P, w], bf, tag="sq", name=f"sq_{off}_{k}")
                nc.gpsimd.tensor_mul(sq_c[:], xk[k][:, sl], xk[k][:, sl])
                nc.tensor.matmul(ps_b[:], ones_k[:], sq_c[:],
                                 start=(k == 0), stop=(k == KH - 1))
            nc.vector.tensor_copy(sumsq[:, sl], ps_b[:])
            ps_l = ps_small.tile([E, w], f32, tag="pss", name=f"ps_lg{off}")
            for k in range(KH):
                nc.tensor.matmul(ps_l[:], wg_sb[:, k, :], xk[k][:, sl],
                                 start=(k == 0), stop=(k == KH - 1))
            nc.vector.tensor_copy(l_sb[:, sl], ps_l[:])

            # LN stats; mean stays unnormalized (sums), 1/H is folded into
            # the apply step.  var = (sumsq - sums^2/H)/H via ACT scale.
            nc.vector.scalar_tensor_tensor(varb[:, sl], sums[:, sl], 1.0 / H,
                                           sums[:, sl], OP.mult, OP.mult)
            nc.vector.tensor_sub(varb[:, sl], sumsq[:, sl], varb[:, sl])
            nc.scalar.activation(sumsq[:, sl], varb[:, sl], AF.Sqrt,
                                 bias=eps_t[:], scale=1.0 / H)
            nc.vector.reciprocal_approx_accurate(
                out=varb[:, sl], in_=sumsq[:, sl], scratch=comb_row[:, sl])
            nc.gpsimd.partition_broadcast(sums_b[:, sl], sums[0:1, sl], P)
            nc.gpsimd.partition_broadcast(inv_b[:, sl], varb[0:1, sl], P)

            # apply LayerNorm -> hdn (bf16):
            #   t1 = (sums_b/H - x) * lnw * inv ;  hdn = -t1 + lnb
            for k in range(KH):
                t1 = tpool.tile([P, w], f32, tag="t1", name=f"t1_{off}_{k}")
                nc.vector.scalar_tensor_tensor(t1[:], sums_b[:, sl], 1.0 / H,
                                               xk[k][:, sl],
                                               OP.mult, OP.subtract)
                nc.vector.scalar_tensor_tensor(t1[:], t1[:], lnw_sb[:, k:k + 1],
                                               inv_b[:, sl], OP.mult, OP.mult)
                nc.scalar.activation(hdn[k][:, sl], t1[:], AF.Identity,
                                     bias=lnb_sb[:, k:k + 1], scale=-1.0)

            # F-block 0 fc1 -> gelu -> fc2 on this chunk
            for m in range(MF):
                pst = ps1.tile([P, w], f32, tag="ps1", name=f"ps1_0_{m}_{ci}")
                for k in range(KH):
                    nc.tensor.matmul(pst[:], w1t0[k][:, m * P:(m + 1) * P],
                                     hdn[k][:, sl],
                                     start=(k == 0), stop=(k == KH - 1))
                nc.scalar.activation(at0[m][:, sl], pst[:],
                                     AF.Gelu_apprx_tanh,
                                     bias=b1_sb[:, m:m + 1])
            for h in range(KH):
                pst = ps2.tile([P, w], f32, tag="ps2", name=f"ps2_0_{h}_{ci}")
                for k in range(MF):
                    nc.tensor.matmul(pst[:], w2t0[k][:, h * P:(h + 1) * P],
                                     at0[k][:, sl],
                                     start=(k == 0), stop=(k == MF - 1))
                nc.scalar.activation(y_acc[h][:, sl], pst[:], AF.Identity,
                                     bias=0.0)

            # top-2 gate (needed only by the final block's finalize): for l_e
            # in the top-2 set the softmax weight is sigmoid(2*l_e - m1 - m2).
            nc.gpsimd.partition_all_reduce(m1b[:, sl], l_sb[:, sl], E,
                                           bass_isa.ReduceOp.max)
            nc.vector.tensor_tensor(eqm[:, sl], l_sb[:, sl], m1b[:, sl],
                                    OP.is_equal)
            nc.vector.scalar_tensor_tensor(eqm[:, sl], eqm[:, sl], -1e30,
                                           l_sb[:, sl], OP.mult, OP.add)
            nc.gpsimd.partition_all_reduce(m2t[:, sl], eqm[:, sl], E,
                                           bass_isa.ReduceOp.max)  # m2
            nc.vector.tensor_add(m1b[:, sl], m1b[:, sl], m2t[:, sl])  # m1+m2
            nc.vector.scalar_tensor_tensor(l_sb[:, sl], l_sb[:, sl], 2.0,
                                           m1b[:, sl], OP.mult, OP.subtract)
            nc.scalar.activation(l_sb[:, sl], l_sb[:, sl], AF.Sigmoid)
            nc.vector.tensor_scalar_mul(l_sb[:, sl], l_sb[:, sl], al_sb[:])
            ps_c = ps_small.tile([1, w], f32, tag="pss", name=f"ps_cmb{off}")
            nc.tensor.matmul(ps_c[:], he_sb[:], l_sb[:, sl],
                             start=True, stop=True)
            nc.vector.tensor_copy(comb_row[:, sl], ps_c[:])
            nc.gpsimd.partition_broadcast(comb_b[:, sl], comb_row[0:1, sl], P)

        # ---- Phase D: remaining F blocks.  Middle blocks iterate
        # weight-stationary (each lhsT feeds all chunks); the last block
        # iterates per-chunk so the finalize tail is short. ----
        for fb in range(1, NFB):
            w1t = []
            w2t = []
            for k in range(KH):
                w1k = w1pool.tile([P, FB], bf, tag="w1", name=f"w1_{fb}_{k}")
                nc.sync.dma_start(
                    w1k[:], d_w1.ap()[k * P:(k + 1) * P, fb * FB:(fb + 1) * FB])
                w1t.append(w1k)
            for k in range(MF):
                w2k = w2pool.tile([P, H], bf, tag="w2", name=f"w2_{fb}_{k}")
                r0 = fb * FB + k * P
                nc.sync.dma_start(w2k[:], d_w2.ap()[r0:r0 + P, :])
                w2t.append(w2k)

            at = [apool.tile([P, C], bf, tag="acts", name=f"a_{fb}_{m}")
                  for m in range(MF)]
            if fb == NFB - 1:
                ci_groups = [[ci] for ci in range(len(chunks))]
            else:
                ci_groups = [list(range(len(chunks)))]

            for cig in ci_groups:
                for m in range(MF):
                    psg = {ci: ps1.tile([P, chunks[ci][1]], f32, tag="ps1",
                                        name=f"ps1_{fb}_{m}_{ci}")
                           for ci in cig}
                    for k in range(KH):
                        lhsT = w1t[k][:, m * P:(m + 1) * P]
                        for ci in cig:
                            off, w = chunks[ci]
                            nc.tensor.matmul(psg[ci][:], lhsT,
                                             hdn[k][:, off:off + w],
                                             start=(k == 0), stop=(k == KH - 1))
                    fcol = fb * MF + m
                    for ci in cig:
                        off, w = chunks[ci]
                        nc.scalar.activation(at[m][:, off:off + w], psg[ci][:],
                                             AF.Gelu_apprx_tanh,
                                             bias=b1_sb[:, fcol:fcol + 1])
                for h in range(KH):
                    psg = {ci: ps2.tile([P, chunks[ci][1]], f32, tag="ps2",
                                        name=f"ps2_{fb}_{h}_{ci}")
                           for ci in cig}
                    for k in range(MF):
                        lhsT = w2t[k][:, h * P:(h + 1) * P]
                        for ci in cig:
                            off, w = chunks[ci]
                            nc.tensor.matmul(psg[ci][:], lhsT,
                                             at[k][:, off:off + w],
                                             start=(k == 0), stop=(k == MF - 1))
                    for ci in cig:
                        off, w = chunks[ci]
                        if fb < NFB - 1:
                            nc.vector.tensor_add(y_acc[h][:, off:off + w],
                                                 y_acc[h][:, off:off + w],
                                                 psg[ci][:])
                        else:
                            # fused finalize: y = (psum + b2) + y_acc, then
                            # scale by the gate weight and store this chunk
                            nc.vector.scalar_tensor_tensor(
                                y_acc[h][:, off:off + w], psg[ci][:],
                                b2_sb[:, h:h + 1], y_acc[h][:, off:off + w],
                                OP.add, OP.add)
                            nc.vector.tensor_mul(y_acc[h][:, off:off + w],
                                                 y_acc[h][:, off:off + w],
                                                 comb_b[:, off:off + w])
                            nc.sync.dma_start(
                                d_y.ap()[h * P:(h + 1) * P, off:off + w],
                                y_acc[h][:, off:off + w])

    nc.compile()
    _BUILD_CACHE[C] = nc
    return nc


def _prepare(x, Wg, alpha, ln_w, ln_b, fc1_w, fc1_b, fc2_w, fc2_b):
    """Host-side routing + per-core input construction."""
    bfnp = ml_dtypes.bfloat16
    xf = np.asarray(x, np.float32).reshape(T, H)
    Wg = np.asarray(Wg, np.float32)
    alpha = np.asarray(alpha, np.float32)
    ln_w = np.asarray(ln_w, np.float32)
    ln_b = np.asarray(ln_b, np.float32)
    fc1_w = np.asarray(fc1_w, np.float32)
    fc1_b = np.asarray(fc1_b, np.float32)
    fc2_w = np.asarray(fc2_w, np.float32)
    fc2_b = np.asarray(fc2_b, np.float32)

    logits = xf @ Wg
    order = np.argsort(-logits, axis=1, kind="stable")
    top2 = order[:, :TOP_K]
    sel = np.zeros((T, E), dtype=bool)
    sel[np.arange(T)[:, None], top2] = True
    idx = [np.nonzero(sel[:, e])[0] for e in range(E)]

    maxc = max(len(i) for i in idx)
    C = max(512, 128 * ((maxc + 127) // 128))

    wg_bf = Wg.astype(bfnp)
    eye = np.eye(E, dtype=np.float32)
    in_maps = []
    for e in range(E):
        n = len(idx[e])
        xg = np.zeros((C, H), np.float32)
        xg[:n] = xf[idx[e]]
        in_maps.append({
            "xgT": np.ascontiguousarray(xg.T).astype(bfnp),
            "w1": fc1_w[e].astype(bfnp),
            "w2": fc2_w[e].astype(bfnp),
            "wg": wg_bf,
            "b1": fc1_b[e].reshape(F, 1).copy(),
            "b2": fc2_b[e].reshape(H, 1).copy(),
            "lnw": ln_w[e].reshape(H, 1).copy(),
            "lnb": ln_b[e].reshape(H, 1).copy(),
            "he": np.ascontiguousarray(eye[:, e:e + 1]),
            "alpha8": np.full((E, 1), alpha[e], np.float32),
        })
    return in_maps, idx, C


def _kernel_impl(inputs, trace=False, trace_cores=None):
    from concourse import bass_utils

    in_maps, idx, C = _prepare(**inputs)
    nc = _build(C)
    res = bass_utils.run_bass_kernel_spmd(
        nc, in_maps, core_ids=list(range(E)),
        trace=trace, trace_cores=trace_cores)

    out = np.zeros((T, H), np.float32)
    for e in range(E):
        yt = np.asarray(res.results[e]["ytT"], np.float32)  # [H, C]
        n = len(idx[e])
        out[idx[e]] += yt.T[:n]
    return out.reshape(B, S, H), res


def kernel(**inputs):
    out, _ = _kernel_impl(inputs)
    return out


# revision 16
# speedup vs baseline: 1.3839x; 1.3839x over previous
"""Trainium2 Bass kernel for a top-2 gated MoE layer (8 experts, H=1024, F=4096).

Strategy (expert parallelism across the 8 NeuronCores):
  - Host computes the top-2 routing (argsort of the fp32 gate logits) and
    gathers each expert's tokens into a padded, transposed activation block
    xgT [H, C] (C = padded per-expert capacity).  All heavy math runs on
    device; the host only shards/gathers.
  - Each core runs one expert: gate logits + top-2 softmax weights are
    recomputed on device from its gathered tokens, LayerNorm + fc1 + gelu +
    fc2 + bias + gate scaling all happen on device (matmuls in bf16 with
    fp32 PSUM accumulation, LN statistics/scalars in fp32).
  - Host scatter-adds the per-expert outputs back into the full [B,S,H]
    tensor.

Self-contained: shapes are hardcoded from the problem spec.
"""

import numpy as np
import ml_dtypes
from contextlib import ExitStack

TOP_K = 2
LN_EPS = 1e-5
B, S, H, E, F = 2, 2048, 1024, 8, 4096
T = B * S
P = 128
KH = H // P          # 8 H-tiles
FB = 1024            # F block size
NFB = F // FB        # 4 blocks
MF = FB // P         # 8 F-tiles per block

_BUILD_CACHE = {}


def _chunks(C):
    # Small first chunk so the LN -> fc1 pipeline fills quickly.
    out = []
    off = 0
    if C >= 768:
        out.append((0, 256))
        off = 256
    while C - off > 512:
        out.append((off, 512))
        off += 512
    if C - off:
        out.append((off, C - off))
    return out


def _build(C):
    """Build + compile the single-core Bass program (SPMD across 8 cores)."""
    if C in _BUILD_CACHE:
        return _BUILD_CACHE[C]

    import concourse.bass as bass  # noqa: F401
    import concourse.tile as tile
    import concourse.mybir as mybir
    from concourse import bacc, bass_isa

    bf = mybir.dt.bfloat16
    f32 = mybir.dt.float32
    AF = mybir.ActivationFunctionType
    OP = mybir.AluOpType

    nc = bacc.Bacc("TRN2", target_bir_lowering=False, debug=False, num_devices=8)

    d_xgT = nc.dram_tensor("xgT", [H, C], bf, kind="ExternalInput")
    d_w1 = nc.dram_tensor("w1", [H, F], bf, kind="ExternalInput")
    d_w2 = nc.dram_tensor("w2", [F, H], bf, kind="ExternalInput")
    d_wg = nc.dram_tensor("wg", [H, E], bf, kind="ExternalInput")
    d_b1 = nc.dram_tensor("b1", [F, 1], f32, kind="ExternalInput")
    d_b2 = nc.dram_tensor("b2", [H, 1], f32, kind="ExternalInput")
    d_lnw = nc.dram_tensor("lnw", [H, 1], f32, kind="ExternalInput")
    d_lnb = nc.dram_tensor("lnb", [H, 1], f32, kind="ExternalInput")
    d_he = nc.dram_tensor("he", [E, 1], f32, kind="ExternalInput")
    d_al = nc.dram_tensor("alpha8", [E, 1], f32, kind="ExternalInput")
    d_y = nc.dram_tensor("ytT", [H, C], f32, kind="ExternalOutput")

    chunks = _chunks(C)

    with tile.TileContext(nc) as tc, ExitStack() as ctx:
        const = ctx.enter_context(tc.tile_pool(name="const", bufs=1))
        gpool = ctx.enter_context(tc.tile_pool(name="gate", bufs=1))
        bpool = ctx.enter_context(tc.tile_pool(name="bcast", bufs=1))
        xpool = ctx.enter_context(tc.tile_pool(name="x", bufs=KH))
        sqpool = ctx.enter_context(tc.tile_pool(name="sq", bufs=3))
        tpool = ctx.enter_context(tc.tile_pool(name="t1", bufs=2))
        hpool = ctx.enter_context(tc.tile_pool(name="hdn", bufs=KH))
        w1pool = ctx.enter_context(tc.tile_pool(name="w1", bufs=9))
        w2pool = ctx.enter_context(tc.tile_pool(name="w2", bufs=9))
        apool = ctx.enter_context(tc.tile_pool(name="acts", bufs=8))
        ypool = ctx.enter_context(tc.tile_pool(name="yacc", bufs=KH))
        ps_small = ctx.enter_context(
            tc.tile_pool(name="ps_small", bufs=2, space="PSUM"))
        ps1 = ctx.enter_context(tc.tile_pool(name="ps1", bufs=3, space="PSUM"))
        ps2 = ctx.enter_context(tc.tile_pool(name="ps2", bufs=3, space="PSUM"))

        # ---- constants / small params ----
        ones_k = const.tile([P, 1], bf)
        nc.vector.memset(ones_k, 1.0)
        # PE warm-up: ~5us of junk matmuls trains the HAM clock gate to
        # 2.4 GHz while the first x DMAs are still in flight.
        warm_rhs = const.tile([P, 512], bf)
        nc.vector.memset(warm_rhs, 0.0)
        ps_w = ps_small.tile([1, 512], f32, tag="pss", name="warm")
        for i in range(16):
            nc.tensor.matmul(ps_w[:], ones_k[:], warm_rhs[:],
                             start=True, stop=True)
        wg_sb = const.tile([P, KH, E], bf)
        for k in range(KH):
            nc.sync.dma_start(wg_sb[:, k, :], d_wg.ap()[k * P:(k + 1) * P, :])
        lnw_sb = const.tile([P, KH], f32)
        lnb_sb = const.tile([P, KH], f32)
        b2_sb = const.tile([P, KH], f32)
        for (t_sb, dram) in ((lnw_sb, d_lnw), (lnb_sb, d_lnb), (b2_sb, d_b2)):
            nc.sync.dma_start(
                t_sb[:], dram.ap().rearrange("(t p) o -> p (t o)", p=P))
        b1_sb = const.tile([P, F // P], f32)
        nc.sync.dma_start(
            b1_sb[:], d_b1.ap().rearrange("(t p) o -> p (t o)", p=P))
        he_sb = const.tile([E, 1], f32)
        nc.sync.dma_start(he_sb[:], d_he.ap())
        al_sb = const.tile([E, 1], f32)
        nc.sync.dma_start(al_sb[:], d_al.ap())

        # ---- Phases A-C, pipelined along C-chunks so the PE can start the
        # fc1 matmuls of chunk 0 while later chunks are still in LN/gate ----
        eps_t = gpool.tile([1, 1], f32)
        nc.vector.memset(eps_t, float(LN_EPS))
        xk = [xpool.tile([P, C], bf, tag="xk", name=f"xk{k}")
              for k in range(KH)]
        hdn = [hpool.tile([P, C], bf, tag="hdn", name=f"hdn{k}")
               for k in range(KH)]
        sums = gpool.tile([1, C], f32)
        sumsq = gpool.tile([1, C], f32)
        varb = gpool.tile([1, C], f32)
        l_sb = gpool.tile([E, C], f32)
        m1b = gpool.tile([E, C], f32)
        eqm = gpool.tile([E, C], f32)
        m2t = gpool.tile([E, C], f32)
        comb_row = gpool.tile([1, C], f32)
        sums_b = bpool.tile([P, C], f32)
        inv_b = bpool.tile([P, C], f32)
        comb_b = bpool.tile([P, C], f32)

        # All x slices first (small), then F-block 0 weights: its fc1/fc2 are
        # emitted inside the chunk loop so the (in-order) PE never waits on
        # later chunks' LN pipeline.
        for ci, (off, w) in enumerate(chunks):
            for k in range(KH):
                nc.sync.dma_start(xk[k][:, off:off + w],
                                  d_xgT.ap()[k * P:(k + 1) * P, off:off + w])
        y_acc = [ypool.tile([P, C], f32, tag="yacc", name=f"yacc{h}")
                 for h in range(KH)]
        w1t0, w2t0 = [], []
        for k in range(KH):
            w1k = w1pool.tile([P, FB], bf, tag="w1", name=f"w1_0_{k}")
            nc.sync.dma_start(w1k[:], d_w1.ap()[k * P:(k + 1) * P, 0:FB])
            w1t0.append(w1k)
        for k in range(MF):
            w2k = w2pool.tile([P, H], bf, tag="w2", name=f"w2_0_{k}")
            nc.sync.dma_start(w2k[:], d_w2.ap()[k * P:(k + 1) * P, :])
            w2t0.append(w2k)
        at0 = [apool.tile([P, C], bf, tag="acts", name=f"a_0_{m}")
               for m in range(MF)]

        for ci, (off, w) in enumerate(chunks):
            sl = slice(off, off + w)
            # column sums / sums of squares / gate logits via PE reductions
            # (squares on GpSimd to keep the DVE free for the LN chain)
            ps_a = ps_small.tile([1, w], f32, tag="pss", name=f"ps_sum{off}")
            for k in range(KH):
                nc.tensor.matmul(ps_a[:], ones_k[:], xk[k][:, sl],
                                 start=(k == 0), stop=(k == KH - 1))
            nc.vector.tensor_copy(sums[:, sl], ps_a[:])
            ps_b = ps_small.tile([1, w], f32, tag="pss", name=f"ps_sq{off}")
            for k in range(KH):
                sq_c = sqpool.tile([P, w], bf, tag="sq", name=f"sq_{off}_{k}")
                nc.vector.tensor_mul(sq_c[:], xk[k][:, sl], xk[k][:, sl])
                nc.tensor.matmul(ps_b[:], ones_k[:], sq_c[:],
                                 start=(k == 0), stop=(k == KH - 1))
            nc.vector.tensor_copy(sumsq[:, sl], ps_b[:])
            ps_l = ps_small.tile([E, w], f32, tag="pss", name=f"ps_lg{off}")
            for k in range(KH):
                nc.tensor.matmul(ps_l[:], wg_sb[:, k, :], xk[k][:, sl],
                                 start=(k == 0), stop=(k == KH - 1))
            nc.vector.tensor_copy(l_sb[:, sl], ps_l[:])

            # LN stats; mean stays unnormalized (sums), 1/H is folded into
            # the apply step.  var = (sumsq - sums^2/H)/H via ACT scale.
            nc.vector.scalar_tensor_tensor(varb[:, sl], sums[:, sl], 1.0 / H,
                                           sums[:, sl], OP.mult, OP.mult)
            nc.vector.tensor_sub(varb[:, sl], sumsq[:, sl], varb[:, sl])
            nc.scalar.activation(sumsq[:, sl], varb[:, sl], AF.Sqrt,
                                 bias=eps_t[:], scale=1.0 / H)
            nc.vector.reciprocal_approx_accurate(
                out=varb[:, sl], in_=sumsq[:, sl], scratch=comb_row[:, sl])
            nc.gpsimd.partition_broadcast(sums_b[:, sl], sums[0:1, sl], P)
            nc.gpsimd.partition_broadcast(inv_b[:, sl], varb[0:1, sl], P)

            # apply LayerNorm -> hdn (bf16):
            #   t1 = (sums_b/H - x) * lnw * inv ;  hdn = -t1 + lnb
            for k in range(KH):
                t1 = tpool.tile([P, w], f32, tag="t1", name=f"t1_{off}_{k}")
                nc.vector.scalar_tensor_tensor(t1[:], sums_b[:, sl], 1.0 / H,
                                               xk[k][:, sl],
                                               OP.mult, OP.subtract)
                nc.vector.scalar_tensor_tensor(t1[:], t1[:], lnw_sb[:, k:k + 1],
                                               inv_b[:, sl], OP.mult, OP.mult)
                nc.scalar.activation(hdn[k][:, sl], t1[:], AF.Identity,
                                     bias=lnb_sb[:, k:k + 1], scale=-1.0)

            # F-block 0 fc1 -> gelu -> fc2 on this chunk
            for m in range(MF):
                pst = ps1.tile([P, w], f32, tag="ps1", name=f"ps1_0_{m}_{ci}")
                for k in range(KH):
                    nc.tensor.matmul(pst[:], w1t0[k][:, m * P:(m + 1) * P],
                                     hdn[k][:, sl],
                                     start=(k == 0), stop=(k == KH - 1))
                nc.scalar.activation(at0[m][:, sl], pst[:],
                                     AF.Gelu_apprx_tanh,
                                     bias=b1_sb[:, m:m + 1])
            for h in range(KH):
                pst = ps2.tile([P, w], f32, tag="ps2", name=f"ps2_0_{h}_{ci}")
                for k in range(MF):
                    nc.tensor.matmul(pst[:], w2t0[k][:, h * P:(h + 1) * P],
                                     at0[k][:, sl],
                                     start=(k == 0), stop=(k == MF - 1))
                nc.scalar.activation(y_acc[h][:, sl], pst[:], AF.Identity,
                                     bias=0.0)

            # top-2 gate (needed only by the final block's finalize): for l_e
            # in the top-2 set the softmax weight is sigmoid(2*l_e - m1 - m2).
            nc.gpsimd.partition_all_reduce(m1b[:, sl], l_sb[:, sl], E,
                                           bass_isa.ReduceOp.max)
            nc.vector.tensor_tensor(eqm[:, sl], l_sb[:, sl], m1b[:, sl],
                                    OP.is_equal)
            nc.vector.scalar_tensor_tensor(eqm[:, sl], eqm[:, sl], -1e30,
                                           l_sb[:, sl], OP.mult, OP.add)
            nc.gpsimd.partition_all_reduce(m2t[:, sl], eqm[:, sl], E,
                                           bass_isa.ReduceOp.max)  # m2
            nc.vector.tensor_add(m1b[:, sl], m1b[:, sl], m2t[:, sl])  # m1+m2
            nc.vector.scalar_tensor_tensor(l_sb[:, sl], l_sb[:, sl], 2.0,
                                           m1b[:, sl], OP.mult, OP.subtract)
            nc.scalar.activation(l_sb[:, sl], l_sb[:, sl], AF.Sigmoid)
            nc.vector.tensor_scalar_mul(l_sb[:, sl], l_sb[:, sl], al_sb[:])
            ps_c = ps_small.tile([1, w], f32, tag="pss", name=f"ps_cmb{off}")
            nc.tensor.matmul(ps_c[:], he_sb[:], l_sb[:, sl],
                             start=True, stop=True)
            nc.vector.tensor_copy(comb_row[:, sl], ps_c[:])
            nc.gpsimd.partition_broadcast(comb_b[:, sl], comb_row[0:1, sl], P)

        # ---- Phase D: remaining F blocks.  Middle blocks iterate
        # weight-stationary (each lhsT feeds all chunks); the last block
        # iterates per-chunk so the finalize tail is short. ----
        for fb in range(1, NFB):
            w1t = []
            w2t = []
            for k in range(KH):
                w1k = w1pool.tile([P, FB], bf, tag="w1", name=f"w1_{fb}_{k}")
                nc.sync.dma_start(
                    w1k[:], d_w1.ap()[k * P:(k + 1) * P, fb * FB:(fb + 1) * FB])
                w1t.append(w1k)
            for k in range(MF):
                w2k = w2pool.tile([P, H], bf, tag="w2", name=f"w2_{fb}_{k}")
                r0 = fb * FB + k * P
                nc.sync.dma_start(w2k[:], d_w2.ap()[r0:r0 + P, :])
                w2t.append(w2k)

            at = [apool.tile([P, C], bf, tag="acts", name=f"a_{fb}_{m}")
                  for m in range(MF)]
            if fb == NFB - 1:
                ci_groups = [[ci] for ci in range(len(chunks))]
            else:
                ci_groups = [list(range(len(chunks)))]

            for cig in ci_groups:
                for m in range(MF):
                    psg = {ci: ps1.tile([P, chunks[ci][1]], f32, tag="ps1",
                                        name=f"ps1_{fb}_{m}_{ci}")
                           for ci in cig}
                    for k in range(KH):
                        lhsT = w1t[k][:, m * P:(m + 1) * P]
                        for ci in cig:
                            off, w = chunks[ci]
                            nc.tensor.matmul(psg[ci][:], lhsT,
                                             hdn[k][:, off:off + w],
                                             start=(k == 0), stop=(k == KH - 1))
                    fcol = fb * MF + m
                    for ci in cig:
                        off, w = chunks[ci]
                        nc.scalar.activation(at[m][:, off:off + w], psg[ci][:],
                                             AF.Gelu_apprx_tanh,
                                             bias=b1_sb[:, fcol:fcol + 1])
                for h in range(KH):
                    psg = {ci: ps2.tile([P, chunks[ci][1]], f32, tag="ps2",
                                        name=f"ps2_{fb}_{h}_{ci}")
                           for ci in cig}
                    for k in range(MF):
                        lhsT = w2t[k][:, h * P:(h + 1) * P]
                        for ci in cig:
                            off, w = chunks[ci]
                            nc.tensor.matmul(psg[ci][:], lhsT,
                                             at[k][:, off:off + w],
                                             start=(k == 0), stop=(k == MF - 1))
                    for ci in cig:
                        off, w = chunks[ci]
                        if fb < NFB - 1:
                            nc.vector.tensor_add(y_acc[h][:, off:off + w],
                                                 y_acc[h][:, off:off + w],
                                                 psg[ci][:])
                        else:
                            # fused finalize: y = (psum + b2) + y_acc, then
                            # scale by the gate weight and store this chunk
                            nc.vector.scalar_tensor_tensor(
                                y_acc[h][:, off:off + w], psg[ci][:],
                                b2_sb[:, h:h + 1], y_acc[h][:, off:off + w],
                                OP.add, OP.add)
                            nc.vector.tensor_mul(y_acc[h][:, off:off + w],
                                                 y_acc[h][:, off:off + w],
                                                 comb_b[:, off:off + w])
                            nc.sync.dma_start(
                                d_y.ap()[h * P:(h + 1) * P, off:off + w],
                                y_acc[h][:, off:off + w])

    nc.compile()
    _BUILD_CACHE[C] = nc
    return nc


def _prepare(x, Wg, alpha, ln_w, ln_b, fc1_w, fc1_b, fc2_w, fc2_b):
    """Host-side routing + per-core input construction."""
    bfnp = ml_dtypes.bfloat16
    xf = np.asarray(x, np.float32).reshape(T, H)
    Wg = np.asarray(Wg, np.float32)
    alpha = np.asarray(alpha, np.float32)
    ln_w = np.asarray(ln_w, np.float32)
    ln_b = np.asarray(ln_b, np.float32)
    fc1_w = np.asarray(fc1_w, np.float32)
    fc1_b = np.asarray(fc1_b, np.float32)
    fc2_w = np.asarray(fc2_w, np.float32)
    fc2_b = np.asarray(fc2_b, np.float32)

    logits = xf @ Wg
    order = np.argsort(-logits, axis=1, kind="stable")
    top2 = order[:, :TOP_K]
    sel = np.zeros((T, E), dtype=bool)
    sel[np.arange(T)[:, None], top2] = True
    idx = [np.nonzero(sel[:, e])[0] for e in range(E)]

    maxc = max(len(i) for i in idx)
    C = max(512, 128 * ((maxc + 127) // 128))

    wg_bf = Wg.astype(bfnp)
    eye = np.eye(E, dtype=np.float32)
    in_maps = []
    for e in range(E):
        n = len(idx[e])
        xg = np.zeros((C, H), np.float32)
        xg[:n] = xf[idx[e]]
        in_maps.append({
            "xgT": np.ascontiguousarray(xg.T).astype(bfnp),
            "w1": fc1_w[e].astype(bfnp),
            "w2": fc2_w[e].astype(bfnp),
            "wg": wg_bf,
            "b1": fc1_b[e].reshape(F, 1).copy(),
            "b2": fc2_b[e].reshape(H, 1).copy(),
            "lnw": ln_w[e].reshape(H, 1).copy(),
            "lnb": ln_b[e].reshape(H, 1).copy(),
            "he": np.ascontiguousarray(eye[:, e:e + 1]),
            "alpha8": np.full((E, 1), alpha[e], np.float32),
        })
    return in_maps, idx, C


def _kernel_impl(inputs, trace=False, trace_cores=None):
    from concourse import bass_utils

    in_maps, idx, C = _prepare(**inputs)
    nc = _build(C)
    res = bass_utils.run_bass_kernel_spmd(
        nc, in_maps, core_ids=list(range(E)),
        trace=trace, trace_cores=trace_cores)

    out = np.zeros((T, H), np.float32)
    for e in range(E):
        yt = np.asarray(res.results[e]["ytT"], np.float32)  # [H, C]
        n = len(idx[e])
        out[idx[e]] += yt.T[:n]
    return out.reshape(B, S, H), res


def kernel(**inputs):
    out, _ = _kernel_impl(inputs)
    return out


# revision 17
# speedup vs baseline: 1.3895x; 1.0040x over previous
"""Trainium2 Bass kernel for a top-2 gated MoE layer (8 experts, H=1024, F=4096).

Strategy (expert parallelism across the 8 NeuronCores):
  - Host computes the top-2 routing (argsort of the fp32 gate logits) and
    gathers each expert's tokens into a padded, transposed activation block
    xgT [H, C] (C = padded per-expert capacity).  All heavy math runs on
    device; the host only shards/gathers.
  - Each core runs one expert: gate logits + top-2 softmax weights are
    recomputed on device from its gathered tokens, LayerNorm + fc1 + gelu +
    fc2 + bias + gate scaling all happen on device (matmuls in bf16 with
    fp32 PSUM accumulation, LN statistics/scalars in fp32).
  - Host scatter-adds the per-expert outputs back into the full [B,S,H]
    tensor.

Self-contained: shapes are hardcoded from the problem spec.
"""

import numpy as np
import ml_dtypes
from contextlib import ExitStack

TOP_K = 2
LN_EPS = 1e-5
B, S, H, E, F = 2, 2048, 1024, 8, 4096
T = B * S
P = 128
KH = H // P          # 8 H-tiles
FB = 1024            # F block size
NFB = F // FB        # 4 blocks
MF = FB // P         # 8 F-tiles per block

_BUILD_CACHE = {}


def _chunks(C):
    # Small first chunk so the LN -> fc1 pipeline fills quickly.
    out = []
    off = 0
    if C >= 768:
        out.append((0, 256))
        off = 256
    while C - off > 512:
        out.append((off, 512))
        off += 512
    if C - off:
        out.append((off, C - off))
    return out


def _build(C):
    """Build + compile the single-core Bass program (SPMD across 8 cores)."""
    if C in _BUILD_CACHE:
        return _BUILD_CACHE[C]

    import concourse.bass as bass  # noqa: F401
    import concourse.tile as tile
    import concourse.mybir as mybir
    from concourse import bacc, bass_isa

    bf = mybir.dt.bfloat16
    f32 = mybir.dt.float32
    AF = mybir.ActivationFunctionType
    OP = mybir.AluOpType

    nc = bacc.Bacc("TRN2", target_bir_lowering=False, debug=False, num_devices=8)

    d_xgT = nc.dram_tensor("xgT", [H, C], bf, kind="ExternalInput")
    d_w1 = nc.dram_tensor("w1", [H, F], bf, kind="ExternalInput")
    d_w2 = nc.dram_tensor("w2", [F, H], bf, kind="ExternalInput")
    d_wg = nc.dram_tensor("wg", [H, E], bf, kind="ExternalInput")
    d_b1 = nc.dram_tensor("b1", [F, 1], f32, kind="ExternalInput")
    d_b2 = nc.dram_tensor("b2", [H, 1], f32, kind="ExternalInput")
    d_lnw = nc.dram_tensor("lnw", [H, 1], f32, kind="ExternalInput")
    d_lnb = nc.dram_tensor("lnb", [H, 1], f32, kind="ExternalInput")
    d_he = nc.dram_tensor("he", [E, 1], f32, kind="ExternalInput")
    d_al = nc.dram_tensor("alpha8", [E, 1], f32, kind="ExternalInput")
    d_y = nc.dram_tensor("ytT", [H, C], f32, kind="ExternalOutput")

    chunks = _chunks(C)

    with tile.TileContext(nc) as tc, ExitStack() as ctx:
        const = ctx.enter_context(tc.tile_pool(name="const", bufs=1))
        gpool = ctx.enter_context(tc.tile_pool(name="gate", bufs=1))
        bpool = ctx.enter_context(tc.tile_pool(name="bcast", bufs=1))
        xpool = ctx.enter_context(tc.tile_pool(name="x", bufs=KH))
        sqpool = ctx.enter_context(tc.tile_pool(name="sq", bufs=8))
        tpool = ctx.enter_context(tc.tile_pool(name="t1", bufs=2))
        hpool = ctx.enter_context(tc.tile_pool(name="hdn", bufs=KH))
        w1pool = ctx.enter_context(tc.tile_pool(name="w1", bufs=9))
        w2pool = ctx.enter_context(tc.tile_pool(name="w2", bufs=9))
        apool = ctx.enter_context(tc.tile_pool(name="acts", bufs=8))
        ypool = ctx.enter_context(tc.tile_pool(name="yacc", bufs=KH))
        ps_small = ctx.enter_context(
            tc.tile_pool(name="ps_small", bufs=2, space="PSUM"))
        ps1 = ctx.enter_context(tc.tile_pool(name="ps1", bufs=3, space="PSUM"))
        ps2 = ctx.enter_context(tc.tile_pool(name="ps2", bufs=3, space="PSUM"))

        # ---- constants / small params ----
        ones_k = const.tile([P, 1], bf)
        nc.vector.memset(ones_k, 1.0)
        # PE warm-up: ~5us of junk matmuls trains the HAM clock gate to
        # 2.4 GHz while the first x DMAs are still in flight.
        warm_rhs = const.tile([P, 512], bf)
        nc.vector.memset(warm_rhs, 0.0)
        ps_w = ps_small.tile([1, 512], f32, tag="pss", name="warm")
        for i in range(24):
            nc.tensor.matmul(ps_w[:], ones_k[:], warm_rhs[:],
                             start=True, stop=True)
        wg_sb = const.tile([P, KH, E], bf)
        for k in range(KH):
            nc.sync.dma_start(wg_sb[:, k, :], d_wg.ap()[k * P:(k + 1) * P, :])
        lnw_sb = const.tile([P, KH], f32)
        lnb_sb = const.tile([P, KH], f32)
        b2_sb = const.tile([P, KH], f32)
        for (t_sb, dram) in ((lnw_sb, d_lnw), (lnb_sb, d_lnb), (b2_sb, d_b2)):
            nc.sync.dma_start(
                t_sb[:], dram.ap().rearrange("(t p) o -> p (t o)", p=P))
        b1_sb = const.tile([P, F // P], f32)
        nc.sync.dma_start(
            b1_sb[:], d_b1.ap().rearrange("(t p) o -> p (t o)", p=P))
        he_sb = const.tile([E, 1], f32)
        nc.sync.dma_start(he_sb[:], d_he.ap())
        al_sb = const.tile([E, 1], f32)
        nc.sync.dma_start(al_sb[:], d_al.ap())

        # ---- Phases A-C, pipelined along C-chunks so the PE can start the
        # fc1 matmuls of chunk 0 while later chunks are still in LN/gate ----
        eps_t = gpool.tile([1, 1], f32)
        nc.vector.memset(eps_t, float(LN_EPS))
        xk = [xpool.tile([P, C], bf, tag="xk", name=f"xk{k}")
              for k in range(KH)]
        hdn = [hpool.tile([P, C], bf, tag="hdn", name=f"hdn{k}")
               for k in range(KH)]
        sums = gpool.tile([1, C], f32)
        sumsq = gpool.tile([1, C], f32)
        varb = gpool.tile([1, C], f32)
        l_sb = gpool.tile([E, C], f32)
        m1b = gpool.tile([E, C], f32)
        eqm = gpool.tile([E, C], f32)
        m2t = gpool.tile([E, C], f32)
        comb_row = gpool.tile([1, C], f32)
        sums_b = bpool.tile([P, C], f32)
        inv_b = bpool.tile([P, C], f32)
        comb_b = bpool.tile([P, C], f32)

        # All x slices first (small), then F-block 0 weights: its fc1/fc2 are
        # emitted inside the chunk loop so the (in-order) PE never waits on
        # later chunks' LN pipeline.
        for ci, (off, w) in enumerate(chunks):
            for k in range(KH):
                nc.sync.dma_start(xk[k][:, off:off + w],
                                  d_xgT.ap()[k * P:(k + 1) * P, off:off + w])
        sq_t = {}
        for ci, (off, w) in enumerate(chunks):
            for k in range(KH):
                sq_c = sqpool.tile([P, w], bf, tag="sq", name=f"sq_{off}_{k}")
                nc.vector.tensor_mul(sq_c[:], xk[k][:, off:off + w],
                                     xk[k][:, off:off + w])
                sq_t[(ci, k)] = sq_c
        y_acc = [ypool.tile([P, C], f32, tag="yacc", name=f"yacc{h}")
                 for h in range(KH)]
        w1t0, w2t0 = [], []
        for k in range(KH):
            w1k = w1pool.tile([P, FB], bf, tag="w1", name=f"w1_0_{k}")
            nc.sync.dma_start(w1k[:], d_w1.ap()[k * P:(k + 1) * P, 0:FB])
            w1t0.append(w1k)
        for k in range(MF):
            w2k = w2pool.tile([P, H], bf, tag="w2", name=f"w2_0_{k}")
            nc.sync.dma_start(w2k[:], d_w2.ap()[k * P:(k + 1) * P, :])
            w2t0.append(w2k)
        at0 = [apool.tile([P, C], bf, tag="acts", name=f"a_0_{m}")
               for m in range(MF)]

        for ci, (off, w) in enumerate(chunks):
            sl = slice(off, off + w)
            # column sums / sums of squares / gate logits via PE reductions
            # (squares on GpSimd to keep the DVE free for the LN chain)
            ps_a = ps_small.tile([1, w], f32, tag="pss", name=f"ps_sum{off}")
            for k in range(KH):
                nc.tensor.matmul(ps_a[:], ones_k[:], xk[k][:, sl],
                                 start=(k == 0), stop=(k == KH - 1))
            nc.vector.tensor_copy(sums[:, sl], ps_a[:])
            ps_b = ps_small.tile([1, w], f32, tag="pss", name=f"ps_sq{off}")
            for k in range(KH):
                nc.tensor.matmul(ps_b[:], ones_k[:], sq_t[(ci, k)][:],
                                 start=(k == 0), stop=(k == KH - 1))
            nc.vector.tensor_copy(sumsq[:, sl], ps_b[:])
            ps_l = ps_small.tile([E, w], f32, tag="pss", name=f"ps_lg{off}")
            for k in range(KH):
                nc.tensor.matmul(ps_l[:], wg_sb[:, k, :], xk[k][:, sl],
                                 start=(k == 0), stop=(k == KH - 1))
            nc.vector.tensor_copy(l_sb[:, sl], ps_l[:])

            # LN stats; mean stays unnormalized (sums), 1/H is folded into
            # the apply step.  var = (sumsq - sums^2/H)/H via ACT scale.
            nc.vector.scalar_tensor_tensor(varb[:, sl], sums[:, sl], 1.0 / H,
                                           sums[:, sl], OP.mult, OP.mult)
            nc.vector.tensor_sub(varb[:, sl], sumsq[:, sl], varb[:, sl])
            nc.scalar.activation(sumsq[:, sl], varb[:, sl], AF.Sqrt,
                                 bias=eps_t[:], scale=1.0 / H)
            nc.vector.reciprocal_approx_fast(out=varb[:, sl],
                                             in_=sumsq[:, sl])
            nc.gpsimd.partition_broadcast(sums_b[:, sl], sums[0:1, sl], P)
            nc.gpsimd.partition_broadcast(inv_b[:, sl], varb[0:1, sl], P)

            # apply LayerNorm -> hdn (bf16):
            #   t1 = (sums_b/H - x) * lnw * inv ;  hdn = -t1 + lnb
            for k in range(KH):
                t1 = tpool.tile([P, w], f32, tag="t1", name=f"t1_{off}_{k}")
                nc.vector.scalar_tensor_tensor(t1[:], sums_b[:, sl], 1.0 / H,
                                               xk[k][:, sl],
                                               OP.mult, OP.subtract)
                nc.vector.scalar_tensor_tensor(t1[:], t1[:], lnw_sb[:, k:k + 1],
                                               inv_b[:, sl], OP.mult, OP.mult)
                nc.scalar.activation(hdn[k][:, sl], t1[:], AF.Identity,
                                     bias=lnb_sb[:, k:k + 1], scale=-1.0)

            # F-block 0 fc1 -> gelu -> fc2 on this chunk
            for m in range(MF):
                pst = ps1.tile([P, w], f32, tag="ps1", name=f"ps1_0_{m}_{ci}")
                for k in range(KH):
                    nc.tensor.matmul(pst[:], w1t0[k][:, m * P:(m + 1) * P],
                                     hdn[k][:, sl],
                                     start=(k == 0), stop=(k == KH - 1))
                nc.scalar.activation(at0[m][:, sl], pst[:],
                                     AF.Gelu_apprx_tanh,
                                     bias=b1_sb[:, m:m + 1])
            for h in range(KH):
                pst = ps2.tile([P, w], f32, tag="ps2", name=f"ps2_0_{h}_{ci}")
                for k in range(MF):
                    nc.tensor.matmul(pst[:], w2t0[k][:, h * P:(h + 1) * P],
                                     at0[k][:, sl],
                                     start=(k == 0), stop=(k == MF - 1))
                nc.scalar.activation(y_acc[h][:, sl], pst[:], AF.Identity,
                                     bias=0.0)

            # top-2 gate (needed only by the final block's finalize): for l_e
            # in the top-2 set the softmax weight is sigmoid(2*l_e - m1 - m2).
            nc.gpsimd.partition_all_reduce(m1b[:, sl], l_sb[:, sl], E,
                                           bass_isa.ReduceOp.max)
            nc.vector.tensor_tensor(eqm[:, sl], l_sb[:, sl], m1b[:, sl],
                                    OP.is_equal)
            nc.vector.scalar_tensor_tensor(eqm[:, sl], eqm[:, sl], -1e30,
                                           l_sb[:, sl], OP.mult, OP.add)
            nc.gpsimd.partition_all_reduce(m2t[:, sl], eqm[:, sl], E,
                                           bass_isa.ReduceOp.max)  # m2
            nc.vector.tensor_add(m1b[:, sl], m1b[:, sl], m2t[:, sl])  # m1+m2
            nc.vector.scalar_tensor_tensor(l_sb[:, sl], l_sb[:, sl], 2.0,
                                           m1b[:, sl], OP.mult, OP.subtract)
            nc.scalar.activation(l_sb[:, sl], l_sb[:, sl], AF.Sigmoid)
            nc.vector.tensor_scalar_mul(l_sb[:, sl], l_sb[:, sl], al_sb[:])
            ps_c = ps_small.tile([1, w], f32, tag="pss", name=f"ps_cmb{off}")
            nc.tensor.matmul(ps_c[:], he_sb[:], l_sb[:, sl],
                             start=True, stop=True)
            nc.vector.tensor_copy(comb_row[:, sl], ps_c[:])
            nc.gpsimd.partition_broadcast(comb_b[:, sl], comb_row[0:1, sl], P)

        # ---- Phase D: remaining F blocks.  Middle blocks iterate
        # weight-stationary (each lhsT feeds all chunks); the last block
        # iterates per-chunk so the finalize tail is short. ----
        for fb in range(1, NFB):
            w1t = []
            w2t = []
            for k in range(KH):
                w1k = w1pool.tile([P, FB], bf, tag="w1", name=f"w1_{fb}_{k}")
                nc.sync.dma_start(
                    w1k[:], d_w1.ap()[k * P:(k + 1) * P, fb * FB:(fb + 1) * FB])
                w1t.append(w1k)
            for k in range(MF):
                w2k = w2pool.tile([P, H], bf, tag="w2", name=f"w2_{fb}_{k}")
                r0 = fb * FB + k * P
                nc.sync.dma_start(w2k[:], d_w2.ap()[r0:r0 + P, :])
                w2t.append(w2k)

            at = [apool.tile([P, C], bf, tag="acts", name=f"a_{fb}_{m}")
                  for m in range(MF)]
            if fb == NFB - 1:
                ci_groups = [[ci] for ci in range(len(chunks))]
            else:
                ci_groups = [list(range(len(chunks)))]

            for cig in ci_groups:
                for m in range(MF):
                    psg = {ci: ps1.tile([P, chunks[ci][1]], f32, tag="ps1",
                                        name=f"ps1_{fb}_{m}_{ci}")
                           for ci in cig}
                    for k in range(KH):
                        lhsT = w1t[k][:, m * P:(m + 1) * P]
                        for ci in cig:
                            off, w = chunks[ci]
                            nc.tensor.matmul(psg[ci][:], lhsT,
                                             hdn[k][:, off:off + w],
                                             start=(k == 0), stop=(k == KH - 1))
                    fcol = fb * MF + m
                    for ci in cig:
                        off, w = chunks[ci]
                        nc.scalar.activation(at[m][:, off:off + w], psg[ci][:],
                                             AF.Gelu_apprx_tanh,
                                             bias=b1_sb[:, fcol:fcol + 1])
                for h in range(KH):
                    psg = {ci: ps2.tile([P, chunks[ci][1]], f32, tag="ps2",
                                        name=f"ps2_{fb}_{h}_{ci}")
                           for ci in cig}
                    for k in range(MF):
                        lhsT = w2t[k][:, h * P:(h + 1) * P]
                        for ci in cig:
                            off, w = chunks[ci]
                            nc.tensor.matmul(psg[ci][:], lhsT,
                                             at[k][:, off:off + w],
                                             start=(k == 0), stop=(k == MF - 1))
                    for ci in cig:
                        off, w = chunks[ci]
                        if fb < NFB - 1:
                            nc.vector.tensor_add(y_acc[h][:, off:off + w],
                                                 y_acc[h][:, off:off + w],
                                                 psg[ci][:])
                        else:
                            # fused finalize: y = (psum + b2) + y_acc, then
                            # scale by the gate weight and store this chunk
                            nc.vector.scalar_tensor_tensor(
                                y_acc[h][:, off:off + w], psg[ci][:],
                                b2_sb[:, h:h + 1], y_acc[h][:, off:off + w],
                                OP.add, OP.add)
                            nc.vector.tensor_mul(y_acc[h][:, off:off + w],
                                                 y_acc[h][:, off:off + w],
                                                 comb_b[:, off:off + w])
                            nc.sync.dma_start(
                                d_y.ap()[h * P:(h + 1) * P, off:off + w],
                                y_acc[h][:, off:off + w])

    nc.compile()
    _BUILD_CACHE[C] = nc
    return nc


def _prepare(x, Wg, alpha, ln_w, ln_b, fc1_w, fc1_b, fc2_w, fc2_b):
    """Host-side routing + per-core input construction."""
    bfnp = ml_dtypes.bfloat16
    xf = np.asarray(x, np.float32).reshape(T, H)
    Wg = np.asarray(Wg, np.float32)
    alpha = np.asarray(alpha, np.float32)
    ln_w = np.asarray(ln_w, np.float32)
    ln_b = np.asarray(ln_b, np.float32)
    fc1_w = np.asarray(fc1_w, np.float32)
    fc1_b = np.asarray(fc1_b, np.float32)
    fc2_w = np.asarray(fc2_w, np.float32)
    fc2_b = np.asarray(fc2_b, np.float32)

    logits = xf @ Wg
    order = np.argsort(-logits, axis=1, kind="stable")
    top2 = order[:, :TOP_K]
    sel = np.zeros((T, E), dtype=bool)
    sel[np.arange(T)[:, None], top2] = True
    idx = [np.nonzero(sel[:, e])[0] for e in range(E)]

    maxc = max(len(i) for i in idx)
    C = max(512, 128 * ((maxc + 127) // 128))

    wg_bf = Wg.astype(bfnp)
    eye = np.eye(E, dtype=np.float32)
    in_maps = []
    for e in range(E):
        n = len(idx[e])
        xg = np.zeros((C, H), np.float32)
        xg[:n] = xf[idx[e]]
        in_maps.append({
            "xgT": np.ascontiguousarray(xg.T).astype(bfnp),
            "w1": fc1_w[e].astype(bfnp),
            "w2": fc2_w[e].astype(bfnp),
            "wg": wg_bf,
            "b1": fc1_b[e].reshape(F, 1).copy(),
            "b2": fc2_b[e].reshape(H, 1).copy(),
            "lnw": ln_w[e].reshape(H, 1).copy(),
            "lnb": ln_b[e].reshape(H, 1).copy(),
            "he": np.ascontiguousarray(eye[:, e:e + 1]),
            "alpha8": np.full((E, 1), alpha[e], np.float32),
        })
    return in_maps, idx, C


def _kernel_impl(inputs, trace=False, trace_cores=None):
    from concourse import bass_utils

    in_maps, idx, C = _prepare(**inputs)
    nc = _build(C)
    res = bass_utils.run_bass_kernel_spmd(
        nc, in_maps, core_ids=list(range(E)),
        trace=trace, trace_cores=trace_cores)

    out = np.zeros((T, H), np.float32)
    for e in range(E):
        yt = np.asarray(res.results[e]["ytT"], np.float32)  # [H, C]
        n = len(idx[e])
        out[idx[e]] += yt.T[:n]
    return out.reshape(B, S, H), res


def kernel(**inputs):
    out, _ = _kernel_impl(inputs)
    return out


# revision 18
# speedup vs baseline: 1.4118x; 1.0161x over previous
"""Trainium2 Bass kernel for a top-2 gated MoE layer (8 experts, H=1024, F=4096).

Strategy (expert parallelism across the 8 NeuronCores):
  - Host computes the top-2 routing (argsort of the fp32 gate logits) and
    gathers each expert's tokens into a padded, transposed activation block
    xgT [H, C] (C = padded per-expert capacity).  All heavy math runs on
    device; the host only shards/gathers.
  - Each core runs one expert: gate logits + top-2 softmax weights are
    recomputed on device from its gathered tokens, LayerNorm + fc1 + gelu +
    fc2 + bias + gate scaling all happen on device (matmuls in bf16 with
    fp32 PSUM accumulation, LN statistics/scalars in fp32).
  - Host scatter-adds the per-expert outputs back into the full [B,S,H]
    tensor.

Self-contained: shapes are hardcoded from the problem spec.
"""

import numpy as np
import ml_dtypes
from contextlib import ExitStack

TOP_K = 2
LN_EPS = 1e-5
B, S, H, E, F = 2, 2048, 1024, 8, 4096
T = B * S
P = 128
KH = H // P          # 8 H-tiles
FB = 1024            # F block size
NFB = F // FB        # 4 blocks
MF = FB // P         # 8 F-tiles per block

_BUILD_CACHE = {}


def _chunks(C):
    # Small first chunk so the LN -> fc1 pipeline fills quickly.
    out = []
    off = 0
    if C >= 768:
        out.append((0, 256))
        off = 256
    while C - off > 512:
        out.append((off, 512))
        off += 512
    if C - off:
        out.append((off, C - off))
    return out


def _build(C):
    """Build + compile the single-core Bass program (SPMD across 8 cores)."""
    if C in _BUILD_CACHE:
        return _BUILD_CACHE[C]

    import concourse.bass as bass  # noqa: F401
    import concourse.tile as tile
    import concourse.mybir as mybir
    from concourse import bacc, bass_isa

    bf = mybir.dt.bfloat16
    f32 = mybir.dt.float32
    AF = mybir.ActivationFunctionType
    OP = mybir.AluOpType

    nc = bacc.Bacc("TRN2", target_bir_lowering=False, debug=False, num_devices=8)

    d_xgT = nc.dram_tensor("xgT", [H, C], bf, kind="ExternalInput")
    d_w1 = nc.dram_tensor("w1", [H, F], bf, kind="ExternalInput")
    d_w2 = nc.dram_tensor("w2", [F, H], bf, kind="ExternalInput")
    d_wg = nc.dram_tensor("wg", [H, E], bf, kind="ExternalInput")
    d_b1 = nc.dram_tensor("b1", [F, 1], f32, kind="ExternalInput")
    d_b2 = nc.dram_tensor("b2", [H, 1], f32, kind="ExternalInput")
    d_lnw = nc.dram_tensor("lnw", [H, 1], f32, kind="ExternalInput")
    d_lnb = nc.dram_tensor("lnb", [H, 1], f32, kind="ExternalInput")
    d_he = nc.dram_tensor("he", [E, 1], f32, kind="ExternalInput")
    d_al = nc.dram_tensor("alpha8", [E, 1], f32, kind="ExternalInput")
    d_y = nc.dram_tensor("ytT", [H, C], f32, kind="ExternalOutput")

    chunks = _chunks(C)

    with tile.TileContext(nc) as tc, ExitStack() as ctx:
        const = ctx.enter_context(tc.tile_pool(name="const", bufs=1))
        gpool = ctx.enter_context(tc.tile_pool(name="gate", bufs=1))
        bpool = ctx.enter_context(tc.tile_pool(name="bcast", bufs=1))
        xpool = ctx.enter_context(tc.tile_pool(name="x", bufs=1))
        sqpool = ctx.enter_context(tc.tile_pool(name="sq", bufs=8))
        tpool = ctx.enter_context(tc.tile_pool(name="t1", bufs=2))
        hpool = ctx.enter_context(tc.tile_pool(name="hdn", bufs=KH))
        w1pool = ctx.enter_context(tc.tile_pool(name="w1", bufs=1))
        w2pool = ctx.enter_context(tc.tile_pool(name="w2", bufs=1))
        apool = ctx.enter_context(tc.tile_pool(name="acts", bufs=8))
        ypool = ctx.enter_context(tc.tile_pool(name="yacc", bufs=1))
        ps_small = ctx.enter_context(
            tc.tile_pool(name="ps_small", bufs=2, space="PSUM"))
        ps1 = ctx.enter_context(tc.tile_pool(name="ps1", bufs=3, space="PSUM"))
        ps2 = ctx.enter_context(tc.tile_pool(name="ps2", bufs=3, space="PSUM"))

        # ---- constants / small params ----
        ones_k = const.tile([P, 1], bf)
        nc.vector.memset(ones_k, 1.0)
        # PE warm-up: ~5us of junk matmuls trains the HAM clock gate to
        # 2.4 GHz while the first x DMAs are still in flight.
        warm_rhs = const.tile([P, 512], bf)
        nc.vector.memset(warm_rhs, 0.0)
        ps_w = ps_small.tile([1, 512], f32, tag="pss", name="warm")
        for i in range(24):
            nc.tensor.matmul(ps_w[:], ones_k[:], warm_rhs[:],
                             start=True, stop=True)
        wg_sb = const.tile([P, KH, E], bf)
        nc.gpsimd.dma_start(wg_sb[:], d_wg.ap().rearrange("(k p) e -> p k e",
                                                          p=P))
        lnw_sb = const.tile([P, KH], f32)
        lnb_sb = const.tile([P, KH], f32)
        b2_sb = const.tile([P, KH], f32)
        for (t_sb, dram) in ((lnw_sb, d_lnw), (lnb_sb, d_lnb), (b2_sb, d_b2)):
            nc.gpsimd.dma_start(
                t_sb[:], dram.ap().rearrange("(t p) o -> p (t o)", p=P))
        b1_sb = const.tile([P, F // P], f32)
        nc.gpsimd.dma_start(
            b1_sb[:], d_b1.ap().rearrange("(t p) o -> p (t o)", p=P))
        he_sb = const.tile([E, 1], f32)
        nc.gpsimd.dma_start(he_sb[:], d_he.ap())
        al_sb = const.tile([E, 1], f32)
        nc.gpsimd.dma_start(al_sb[:], d_al.ap())

        # ---- Phases A-C, pipelined along C-chunks so the PE can start the
        # fc1 matmuls of chunk 0 while later chunks are still in LN/gate ----
        eps_t = gpool.tile([1, 1], f32)
        nc.vector.memset(eps_t, float(LN_EPS))
        xbig = xpool.tile([P, KH, C], bf, tag="xk", name="xbig")
        xk = [xbig[:, k, :] for k in range(KH)]
        d_xr = d_xgT.ap().rearrange("(k p) c -> p k c", p=P)
        w0 = chunks[0][1]
        nc.sync.dma_start(xbig[:, :, 0:w0], d_xr[:, :, 0:w0])
        nc.sync.dma_start(xbig[:, :, w0:C], d_xr[:, :, w0:C])
        hdn = [hpool.tile([P, C], bf, tag="hdn", name=f"hdn{k}")
               for k in range(KH)]
        sums = gpool.tile([1, C], f32)
        sumsq = gpool.tile([1, C], f32)
        varb = gpool.tile([1, C], f32)
        l_sb = gpool.tile([E, C], f32)
        m1b = gpool.tile([E, C], f32)
        eqm = gpool.tile([E, C], f32)
        m2t = gpool.tile([E, C], f32)
        comb_row = gpool.tile([1, C], f32)
        sums_b = bpool.tile([P, C], f32)
        inv_b = bpool.tile([P, C], f32)
        comb_b = bpool.tile([P, C], f32)

        sq_t = {}
        for ci, (off, w) in enumerate(chunks):
            for k in range(KH):
                sq_c = sqpool.tile([P, w], bf, tag="sq", name=f"sq_{off}_{k}")
                nc.vector.tensor_mul(sq_c[:], xk[k][:, off:off + w],
                                     xk[k][:, off:off + w])
                sq_t[(ci, k)] = sq_c
        ybig = ypool.tile([P, KH, C], f32, tag="yacc", name="ybig")
        y_acc = [ybig[:, h, :] for h in range(KH)]
        d_yr = d_y.ap().rearrange("(k p) c -> p k c", p=P)

        def load_w_block(fb):
            w1blk = w1pool.tile([P, KH, FB], bf, tag="w1", name=f"w1_{fb}")
            nc.sync.dma_start(
                w1blk[:],
                d_w1.ap()[:, fb * FB:(fb + 1) * FB].rearrange(
                    "(k p) f -> p k f", p=P))
            w2blk = w2pool.tile([P, MF, H], bf, tag="w2", name=f"w2_{fb}")
            nc.sync.dma_start(
                w2blk[:],
                d_w2.ap()[fb * FB:(fb + 1) * FB, :].rearrange(
                    "(k p) h -> p k h", p=P))
            return ([w1blk[:, k, :] for k in range(KH)],
                    [w2blk[:, k, :] for k in range(MF)])

        w1t0, w2t0 = load_w_block(0)
        at0 = [apool.tile([P, C], bf, tag="acts", name=f"a_0_{m}")
               for m in range(MF)]

        for ci, (off, w) in enumerate(chunks):
            sl = slice(off, off + w)
            # column sums / sums of squares / gate logits via PE reductions
            # (squares on GpSimd to keep the DVE free for the LN chain)
            ps_a = ps_small.tile([1, w], f32, tag="pss", name=f"ps_sum{off}")
            for k in range(KH):
                nc.tensor.matmul(ps_a[:], ones_k[:], xk[k][:, sl],
                                 start=(k == 0), stop=(k == KH - 1))
            nc.vector.tensor_copy(sums[:, sl], ps_a[:])
            ps_b = ps_small.tile([1, w], f32, tag="pss", name=f"ps_sq{off}")
            for k in range(KH):
                nc.tensor.matmul(ps_b[:], ones_k[:], sq_t[(ci, k)][:],
                                 start=(k == 0), stop=(k == KH - 1))
            nc.vector.tensor_copy(sumsq[:, sl], ps_b[:])
            ps_l = ps_small.tile([E, w], f32, tag="pss", name=f"ps_lg{off}")
            for k in range(KH):
                nc.tensor.matmul(ps_l[:], wg_sb[:, k, :], xk[k][:, sl],
                                 start=(k == 0), stop=(k == KH - 1))
            nc.vector.tensor_copy(l_sb[:, sl], ps_l[:])

            # LN stats; mean stays unnormalized (sums), 1/H is folded into
            # the apply step.  var = (sumsq - sums^2/H)/H via ACT scale.
            nc.vector.scalar_tensor_tensor(varb[:, sl], sums[:, sl], 1.0 / H,
                                           sums[:, sl], OP.mult, OP.mult)
            nc.vector.tensor_sub(varb[:, sl], sumsq[:, sl], varb[:, sl])
            nc.scalar.activation(sumsq[:, sl], varb[:, sl], AF.Sqrt,
                                 bias=eps_t[:], scale=1.0 / H)
            nc.vector.reciprocal_approx_fast(out=varb[:, sl],
                                             in_=sumsq[:, sl])
            nc.gpsimd.partition_broadcast(sums_b[:, sl], sums[0:1, sl], P)
            nc.gpsimd.partition_broadcast(inv_b[:, sl], varb[0:1, sl], P)

            # apply LayerNorm -> hdn (bf16):
            #   t1 = (sums_b/H - x) * lnw * inv ;  hdn = -t1 + lnb
            for k in range(KH):
                t1 = tpool.tile([P, w], f32, tag="t1", name=f"t1_{off}_{k}")
                nc.vector.scalar_tensor_tensor(t1[:], sums_b[:, sl], 1.0 / H,
                                               xk[k][:, sl],
                                               OP.mult, OP.subtract)
                nc.vector.scalar_tensor_tensor(t1[:], t1[:], lnw_sb[:, k:k + 1],
                                               inv_b[:, sl], OP.mult, OP.mult)
                nc.scalar.activation(hdn[k][:, sl], t1[:], AF.Identity,
                                     bias=lnb_sb[:, k:k + 1], scale=-1.0)

            # F-block 0 fc1 -> gelu -> fc2 on this chunk
            for m in range(MF):
                pst = ps1.tile([P, w], f32, tag="ps1", name=f"ps1_0_{m}_{ci}")
                for k in range(KH):
                    nc.tensor.matmul(pst[:], w1t0[k][:, m * P:(m + 1) * P],
                                     hdn[k][:, sl],
                                     start=(k == 0), stop=(k == KH - 1))
                nc.scalar.activation(at0[m][:, sl], pst[:],
                                     AF.Gelu_apprx_tanh,
                                     bias=b1_sb[:, m:m + 1])
            for h in range(KH):
                pst = ps2.tile([P, w], f32, tag="ps2", name=f"ps2_0_{h}_{ci}")
                for k in range(MF):
                    nc.tensor.matmul(pst[:], w2t0[k][:, h * P:(h + 1) * P],
                                     at0[k][:, sl],
                                     start=(k == 0), stop=(k == MF - 1))
                nc.scalar.activation(y_acc[h][:, sl], pst[:], AF.Identity,
                                     bias=0.0)

            # top-2 gate (needed only by the final block's finalize): for l_e
            # in the top-2 set the softmax weight is sigmoid(2*l_e - m1 - m2).
            nc.gpsimd.partition_all_reduce(m1b[:, sl], l_sb[:, sl], E,
                                           bass_isa.ReduceOp.max)
            nc.vector.tensor_tensor(eqm[:, sl], l_sb[:, sl], m1b[:, sl],
                                    OP.is_equal)
            nc.vector.scalar_tensor_tensor(eqm[:, sl], eqm[:, sl], -1e30,
                                           l_sb[:, sl], OP.mult, OP.add)
            nc.gpsimd.partition_all_reduce(m2t[:, sl], eqm[:, sl], E,
                                           bass_isa.ReduceOp.max)  # m2
            nc.vector.tensor_add(m1b[:, sl], m1b[:, sl], m2t[:, sl])  # m1+m2
            nc.vector.scalar_tensor_tensor(l_sb[:, sl], l_sb[:, sl], 2.0,
                                           m1b[:, sl], OP.mult, OP.subtract)
            nc.scalar.activation(l_sb[:, sl], l_sb[:, sl], AF.Sigmoid)
            nc.vector.tensor_scalar_mul(l_sb[:, sl], l_sb[:, sl], al_sb[:])
            ps_c = ps_small.tile([1, w], f32, tag="pss", name=f"ps_cmb{off}")
            nc.tensor.matmul(ps_c[:], he_sb[:], l_sb[:, sl],
                             start=True, stop=True)
            nc.vector.tensor_copy(comb_row[:, sl], ps_c[:])
            nc.gpsimd.partition_broadcast(comb_b[:, sl], comb_row[0:1, sl], P)

        # ---- Phase D: remaining F blocks.  Middle blocks iterate
        # weight-stationary (each lhsT feeds all chunks); the last block
        # iterates per-chunk so the finalize tail is short. ----
        for fb in range(1, NFB):
            w1t, w2t = load_w_block(fb)

            at = [apool.tile([P, C], bf, tag="acts", name=f"a_{fb}_{m}")
                  for m in range(MF)]
            if fb == NFB - 1:
                ci_groups = [[ci] for ci in range(len(chunks))]
            else:
                ci_groups = [list(range(len(chunks)))]

            for cig in ci_groups:
                for m in range(MF):
                    psg = {ci: ps1.tile([P, chunks[ci][1]], f32, tag="ps1",
                                        name=f"ps1_{fb}_{m}_{ci}")
                           for ci in cig}
                    for k in range(KH):
                        lhsT = w1t[k][:, m * P:(m + 1) * P]
                        for ci in cig:
                            off, w = chunks[ci]
                            nc.tensor.matmul(psg[ci][:], lhsT,
                                             hdn[k][:, off:off + w],
                                             start=(k == 0), stop=(k == KH - 1))
                    fcol = fb * MF + m
                    for ci in cig:
                        off, w = chunks[ci]
                        nc.scalar.activation(at[m][:, off:off + w], psg[ci][:],
                                             AF.Gelu_apprx_tanh,
                                             bias=b1_sb[:, fcol:fcol + 1])
                for h in range(KH):
                    psg = {ci: ps2.tile([P, chunks[ci][1]], f32, tag="ps2",
                                        name=f"ps2_{fb}_{h}_{ci}")
                           for ci in cig}
                    for k in range(MF):
                        lhsT = w2t[k][:, h * P:(h + 1) * P]
                        for ci in cig:
                            off, w = chunks[ci]
                            nc.tensor.matmul(psg[ci][:], lhsT,
                                             at[k][:, off:off + w],
                                             start=(k == 0), stop=(k == MF - 1))
                    for ci in cig:
                        off, w = chunks[ci]
                        if fb < NFB - 1:
                            nc.vector.tensor_add(y_acc[h][:, off:off + w],
                                                 y_acc[h][:, off:off + w],
                                                 psg[ci][:])
                        else:
                            # fused finalize: y = (psum + b2) + y_acc, then
                            # scale by the gate weight and store this chunk
                            nc.vector.scalar_tensor_tensor(
                                y_acc[h][:, off:off + w], psg[ci][:],
                                b2_sb[:, h:h + 1], y_acc[h][:, off:off + w],
                                OP.add, OP.add)
                            nc.vector.tensor_mul(y_acc[h][:, off:off + w],
                                                 y_acc[h][:, off:off + w],
                                                 comb_b[:, off:off + w])
                if fb == NFB - 1:
                    for ci in cig:
                        off, w = chunks[ci]
                        nc.sync.dma_start(d_yr[:, :, off:off + w],
                                          ybig[:, :, off:off + w])

    nc.compile()
    _BUILD_CACHE[C] = nc
    return nc


def _prepare(x, Wg, alpha, ln_w, ln_b, fc1_w, fc1_b, fc2_w, fc2_b):
    """Host-side routing + per-core input construction."""
    bfnp = ml_dtypes.bfloat16
    xf = np.asarray(x, np.float32).reshape(T, H)
    Wg = np.asarray(Wg, np.float32)
    alpha = np.asarray(alpha, np.float32)
    ln_w = np.asarray(ln_w, np.float32)
    ln_b = np.asarray(ln_b, np.float32)
    fc1_w = np.asarray(fc1_w, np.float32)
    fc1_b = np.asarray(fc1_b, np.float32)
    fc2_w = np.asarray(fc2_w, np.float32)
    fc2_b = np.asarray(fc2_b, np.float32)

    logits = xf @ Wg
    order = np.argsort(-logits, axis=1, kind="stable")
    top2 = order[:, :TOP_K]
    sel = np.zeros((T, E), dtype=bool)
    sel[np.arange(T)[:, None], top2] = True
    idx = [np.nonzero(sel[:, e])[0] for e in range(E)]

    maxc = max(len(i) for i in idx)
    C = max(512, 128 * ((maxc + 127) // 128))

    wg_bf = Wg.astype(bfnp)
    eye = np.eye(E, dtype=np.float32)
    in_maps = []
    for e in range(E):
        n = len(idx[e])
        xg = np.zeros((C, H), np.float32)
        xg[:n] = xf[idx[e]]
        in_maps.append({
            "xgT": np.ascontiguousarray(xg.T).astype(bfnp),
            "w1": fc1_w[e].astype(bfnp),
            "w2": fc2_w[e].astype(bfnp),
            "wg": wg_bf,
            "b1": fc1_b[e].reshape(F, 1).copy(),
            "b2": fc2_b[e].reshape(H, 1).copy(),
            "lnw": ln_w[e].reshape(H, 1).copy(),
            "lnb": ln_b[e].reshape(H, 1).copy(),
            "he": np.ascontiguousarray(eye[:, e:e + 1]),
            "alpha8": np.full((E, 1), alpha[e], np.float32),
        })
    return in_maps, idx, C


def _kernel_impl(inputs, trace=False, trace_cores=None):
    from concourse import bass_utils

    in_maps, idx, C = _prepare(**inputs)
    nc = _build(C)
    res = bass_utils.run_bass_kernel_spmd(
        nc, in_maps, core_ids=list(range(E)),
        trace=trace, trace_cores=trace_cores)

    out = np.zeros((T, H), np.float32)
    for e in range(E):
        yt = np.asarray(res.results[e]["ytT"], np.float32)  # [H, C]
        n = len(idx[e])
        out[idx[e]] += yt.T[:n]
    return out.reshape(B, S, H), res


def kernel(**inputs):
    out, _ = _kernel_impl(inputs)
    return out
